# revision 19
# baseline (speedup 1.0000x reference)
"""DirectedDualSAGE (2-layer dual-direction GraphSAGE + MLP head) on 8 trn2
NeuronCores via Bass/Tile.

Sharding: nodes (dsts) block-partitioned 6250/core; each core owns all edges
whose dst lies in its shard, for both edge directions.

Per layer, lin_l(mean_j x_j) for BOTH directions is computed as
diag(1/cnt) * A * (x @ [Wl_in|Wl_out]): transform first, bf16 tables with
both directions packed per row (256B rows = the dma_gather element floor).
The segment-mean is a per-edge row gather (dma_gather, one dedicated SWDGE
queue per (direction, src-half) stream) from a DRAM table + prefix-structured
accumulation on the vector engine: dsts are sorted by descending degree so
"round r" (the r-th edge of every dst) occupies slot prefix [0, n_r) and
accumulates with one contiguous tensor_tensor add per gather fragment
(in-dir adds read gathered cols 0:64, out-dir cols 64:128). Gather indices
are int16, so edges are split by src half (< / >= 25000).

Layer 1's tables are built replicated (x is an input -> no communication) in
a PARTITION-MAJOR physical layout (logical row i at physical row
(i%128)*K + i//128, host remaps indices) so the table writes are contiguous
multi-KB runs per partition instead of 256B scatter. Layer 2's table is
built from the local x2 shard and AllGather'ed in ONE collective (both
directions packed), row-major natural layout.

Dense math runs feature-major on the tensor engine; aggregated means
(node-major) are transposed back via PE identity-matmuls accumulating into
the same PSUM as the x @ Wr term, then bias+relu on the scalar engine.

kernel(**inputs) takes full unsharded inputs, returns the full [N] output.
"""
import numpy as np

import concourse.bacc as bacc
import concourse.tile as tile
import concourse.mybir as mybir
from concourse import bass_utils

F32 = mybir.dt.float32
BF16 = mybir.dt.bfloat16
I16 = mybir.dt.int16

N = 50000
NC = 8
NLOC = N // NC            # 6250
NLOCP = 6272              # 49*128
NCH = NLOCP // 128        # 49 chunks
XCOLS = 50088             # 25000 + 49*512 (xt_full padded cols)
HALF = 25000              # src half split
ZHEAD = 128               # zero rows at table head
HROWSP = 25344            # 198*128: L1 half-table rows (partition-major)
KH = HROWSP // 128        # 198
TROWS = ZHEAD + N + 176   # 50304 rows for the AllGather'ed Y2 table
BZERO = ZHEAD + HALF      # 25128: L2 B-half zero idx (y2 row 50128 abs)
SMAX = 2560               # max rows per dma_gather call
QMAP = {("in", "A"): 0, ("in", "B"): 1, ("out", "A"): 2, ("out", "B"): 3}

_CACHE = {}


# ----------------------------------------------------------------- host prep

def _round_up(v, m):
    return (v + m - 1) // m * m


def _l1_remap(logical):
    """Partition-major physical row for L1 half tables."""
    return (logical % 128) * KH + logical // 128


def _per_core_half(src, dst, half_mask):
    out = []
    for c in range(NC):
        m = (dst // NLOC == c) & half_mask
        s = src[m]
        dloc = (dst[m] - c * NLOC).astype(np.int64)
        deg = np.bincount(dloc, minlength=NLOCP).astype(np.int64)
        perm = np.argsort(-deg, kind="stable").astype(np.int64)
        pos = np.empty(NLOCP, dtype=np.int64)
        pos[perm] = np.arange(NLOCP)
        order = np.argsort(dloc, kind="stable")
        sd = dloc[order]
        ss = s[order]
        if len(sd):
            starts = np.r_[0, 1 + np.flatnonzero(np.diff(sd))]
            group_id = np.zeros(len(sd), dtype=np.int64)
            group_id[starts[1:]] = 1
            group_id = np.cumsum(group_id)
            rank = np.arange(len(sd)) - starts[group_id]
        else:
            rank = sd
        slot = pos[sd]
        maxdeg = int(deg.max()) if len(sd) else 0
        rounds = []
        for r in range(maxdeg):
            mr = rank == r
            rounds.append((int(np.count_nonzero(mr)), slot[mr], ss[mr]))
        out.append(dict(deg=deg, pos=pos, rounds=rounds))
    return out


def _cut_groups(NR):
    L = int(sum(NR))
    groups = []  # (stream_off, [(stg_off, acc_slot_off, nrows, r)])
    r, r_off = 0, 0
    off = 0
    while off < L:
        rows = min(SMAX, L - off)
        frags = []
        done = 0
        while done < rows:
            take = min(NR[r] - r_off, rows - done)
            frags.append((done, r_off, take, r))
            done += take
            r_off += take
            if r_off == NR[r]:
                r += 1
                r_off = 0
        groups.append((off, frags))
        off += rows
    return groups


def _preprocess(edge_index_in, edge_index_out):
    plan = {"dirs": {}}
    for dname, ei in (("in", edge_index_in), ("out", edge_index_out)):
        src = ei[0].astype(np.int64)
        dst = ei[1].astype(np.int64)
        dinfo = {"halves": {}, "recip": []}
        for c in range(NC):
            m = dst // NLOC == c
            dloc = dst[m] - c * NLOC
            cnt = np.bincount(dloc, minlength=NLOCP).astype(np.float32)
            dinfo["recip"].append((1.0 / np.maximum(cnt, 1.0)).astype(np.float32))
        for hname, is_a in (("A", True), ("B", False)):
            half_mask = (src < HALF) if is_a else (src >= HALF)
            cores = _per_core_half(src, dst, half_mask)
            nrounds = max(len(ci["rounds"]) for ci in cores)
            NR = []
            for r in range(nrounds):
                mx = max((ci["rounds"][r][0] if r < len(ci["rounds"]) else 0)
                         for ci in cores)
                NR.append(_round_up(max(mx, 1), 128))
            NR[0] = NLOCP  # full first round: copy-initializes the accumulator
            streams1 = []   # layer-1 (partition-major remapped)
            streams2 = []   # layer-2 (natural + ZHEAD, shared-table offsets)
            zi2 = 0 if is_a else BZERO
            for ci in cores:
                parts1, parts2 = [], []
                for r in range(nrounds):
                    v1 = np.zeros(NR[r], dtype=np.int64)  # L1 pad -> phys 0
                    v2 = np.full(NR[r], zi2, dtype=np.int64)
                    if r < len(ci["rounds"]):
                        _, slots, ss = ci["rounds"][r]
                        rel = (ss if is_a else ss - HALF) + ZHEAD
                        v1[slots] = _l1_remap(rel)
                        v2[slots] = rel
                    parts1.append(v1)
                    parts2.append(v2)
                s1 = np.concatenate(parts1)
                s2 = np.concatenate(parts2)
                assert s1.max(initial=0) < 32768 and s2.max(initial=0) < 32768
                streams1.append(s1.astype(np.int16))
                streams2.append(s2.astype(np.int16))
            dinfo["halves"][hname] = dict(
                NR=NR, L=int(sum(NR)), streams1=streams1, streams2=streams2,
                groups=_cut_groups(NR),
                unperm=[ci["pos"].astype(np.int16) for ci in cores], is_a=is_a,
            )
        plan["dirs"][dname] = dinfo
    return plan


def _wrap_idx(idx):
    L = idx.shape[0]
    assert L % 16 == 0
    w = idx.reshape(L // 16, 16).T.astype(np.int16)
    return np.ascontiguousarray(np.tile(w, (8, 1)))


# ------------------------------------------------------------- device program

def _build_program(plan):
    nc = bacc.Bacc("TRN2", target_bir_lowering=False, debug=False,
                   num_devices=NC, num_swdge_queues=4)
    dims = ("in", "out")
    inp = {}

    def dram_in(name, shape, dt=F32):
        inp[name] = nc.dram_tensor(name, list(shape), dt, kind="ExternalInput")
        return inp[name]

    xt_full = dram_in("xt_full", [128, XCOLS])
    xt_loc = dram_in("xt_loc", [128, NLOCP])
    ident = dram_in("ident", [128, 128])
    for li in (1, 2):
        dram_in(f"wl_comb{li}", [128, 128])
        dram_in(f"wr_in{li}", [128, 64])
        dram_in(f"wr_out{li}", [128, 64])
        dram_in(f"bias_pk{li}", [128, 1])
        dram_in(f"wcx{li}", [128, 128])
        dram_in(f"wch{li}", [128, 128])
        dram_in(f"cb{li}", [128, 1])
    dram_in("fw", [128, 1])
    dram_in("fb", [1, 1])
    UPC = NLOCP // 16  # unperm idx cols
    for d in dims:
        dram_in(f"recip_{d}", [128, NCH])
        for h in ("A", "B"):
            L = plan["dirs"][d]["halves"][h]["L"]
            for li in (1, 2):
                dram_in(f"idx{li}_{d}_{h}", [128, L // 16 + UPC], I16)

    # L1 half tables: partition-major [128, KH, 128] bf16 (both dirs packed)
    y1t = {h: nc.dram_tensor(f"y1{h}", [128, KH, 128], BF16, kind="Internal")
           for h in ("A", "B")}
    y2sl = nc.dram_tensor("y2sl", [NLOCP, 128], BF16, kind="Internal")
    y2t = nc.dram_tensor("y2", [TROWS, 128], BF16, kind="Internal",
                         addr_space="Shared")
    accd = {d: nc.dram_tensor(f"accd_{d}", [2, NLOCP, 64], F32, kind="Internal")
            for d in dims}
    out_t = nc.dram_tensor("out", [1, NLOC], F32, kind="ExternalOutput")

    RELU = mybir.ActivationFunctionType.Relu
    COPY = mybir.ActivationFunctionType.Copy

    with tile.TileContext(nc) as tc:
        with tc.tile_pool(name="const", bufs=1) as cpool, \
             tc.tile_pool(name="bigidx", bufs=4) as bigidxp, \
             tc.tile_pool(name="feat", bufs=1) as featp, \
             tc.tile_pool(name="accp", bufs=1) as accp, \
             tc.tile_pool(name="meanp", bufs=1) as meanp, \
             tc.tile_pool(name="stg", bufs=6) as stgp, \
             tc.tile_pool(name="natp", bufs=2) as natp, \
             tc.tile_pool(name="ybld", bufs=2) as ybldp, \
             tc.tile_pool(name="ystg", bufs=2) as ystgp, \
             tc.tile_pool(name="small", bufs=1) as smallp, \
             tc.tile_pool(name="ps", bufs=3, space="PSUM") as psp, \
             tc.tile_pool(name="psf", bufs=1, space="PSUM") as psfp, \
             tc.tile_pool(name="psy", bufs=2, space="PSUM") as psyp:

            def load_const(name, shape, dt=F32):
                t = cpool.tile(list(shape), dt, tag=name, name=f"c_{name}")
                nc.sync.dma_start(t[:], inp[name][tuple(slice(None) for _ in shape)])
                return t

            ident_t = load_const("ident", [128, 128])
            ident_b = cpool.tile([128, 128], BF16, tag="identb")
            nc.vector.tensor_copy(ident_b[:], ident_t[:])
            W = {}
            for li in (1, 2):
                for nm, shp in (("wl_comb", [128, 128]), ("wr_in", [128, 64]),
                                ("wr_out", [128, 64]), ("bias_pk", [128, 1]),
                                ("wcx", [128, 128]), ("wch", [128, 128]),
                                ("cb", [128, 1])):
                    W[f"{nm}{li}"] = load_const(f"{nm}{li}", shp)
            wch_b = {}
            for li in (1, 2):
                wb = cpool.tile([128, 128], BF16, tag=f"wchb{li}")
                nc.vector.tensor_copy(wb[:], W[f"wch{li}"][:])
                wch_b[li] = wb
            fw_t = load_const("fw", [128, 1])
            fw_b = cpool.tile([128, 1], BF16, tag="fwb")
            nc.vector.tensor_copy(fw_b[:], fw_t[:])
            fb_t = load_const("fb", [1, 1])
            recip_t = {d: load_const(f"recip_{d}", [128, NCH]) for d in dims}
            IDXC = max(plan["dirs"][d]["halves"][h]["L"] // 16
                       for d in dims for h in ("A", "B")) + NLOCP // 16
            idx_t = {}

            def load_idx(li, d, h):
                L = plan["dirs"][d]["halves"][h]["L"]
                t = bigidxp.tile([128, IDXC], I16, tag="bigidx",
                                 name=f"bidx{li}_{d}_{h}")
                nc.scalar.dma_start(t[:, 0:L // 16 + NLOCP // 16],
                                    inp[f"idx{li}_{d}_{h}"][:, :])
                idx_t[li, d, h] = t

            zero_b = smallp.tile([128, 128], BF16, tag="zerob")
            nc.vector.memset(zero_b[:], 0.0)

            # L1 tables: zero the pad target (logical row 0 = phys row 0) and
            # the tail region beyond written data (phys rows for logical
            # >= ZHEAD+HALF+88 are never indexed; only row 0 is the pad).
            for h in ("A", "B"):
                nc.sync.dma_start(
                    y1t[h][0:1, 0:1, :].rearrange("p k c -> p (k c)"),
                    zero_b[0:1, 0:128])
            # y2 zero head+tail rows (row-major)
            def zero_rows_y2(start, nrow):
                nc.sync.dma_start(
                    y2t[start:start + nrow, :]
                    .rearrange("(k p) c -> p k c", p=nrow),
                    zero_b[0:nrow, :].rearrange("p (k c) -> p k c", k=1))
            zero_rows_y2(0, 128)
            zero_rows_y2(ZHEAD + N, 128)
            zero_rows_y2(TROWS - 48, 48)

            # ---------------- y1 table build (replicated, partition-major).
            # Loads on scalar (A) / sync (B) HWDGE, PSUM->bf16 copies on the
            # vector engine, table writes on gpsimd SWDGE: no engine carries
            # two chained stages, so the block pipeline flows at DMA rate.
            def build_half_steps(h, col0, ld_eng, w_eng, pspool, pstag,
                                 ystag):
                steps = []

                def mk(t_off):
                    def f():
                        tw = min(512, 25088 - t_off)
                        xs = ybldp.tile([128, 512], F32, tag=f"xs{h}")
                        ld_eng.dma_start(
                            xs[:, 0:tw],
                            xt_full[:, col0 + t_off:col0 + t_off + tw])
                        ps = pspool.tile([128, 512], F32, tag=pstag)
                        for k in range(tw // 128):
                            nc.tensor.matmul(ps[:, 128 * k:128 * (k + 1)],
                                             xs[:, 128 * k:128 * (k + 1)],
                                             W["wl_comb1"][:],
                                             start=True, stop=True)
                        ys = ystgp.tile([128, 512], BF16, tag=ystag)
                        nc.vector.tensor_copy(ys[:, 0:tw], ps[:, 0:tw])
                        kk = tw // 128
                        k0 = t_off // 128 + 1  # +1: ZHEAD shifts logical rows
                        w_eng.dma_start(
                            y1t[h][:, k0:k0 + kk, :],
                            ys[:, 0:tw].rearrange("p (k c) -> p k c", k=kk))
                    return f
                for t_off in range(0, 25088, 512):
                    steps.append(mk(t_off))
                return steps

            # ---------------- shared helpers
            hidx = {"A": 0, "B": 1}
            _qctr = [0]

            def next_queue():
                q = _qctr[0] % 4
                _qctr[0] += 1
                return q
            col0 = {"in": 0, "out": 64}

            def gather_half_steps(li, d, h, tab_ap):
                """Per-group closures for one (direction, half) stream; the
                final closure writes accd. Interleave across streams to avoid
                sequencer head-of-line blocking on one queue's ring."""
                hinfo = plan["dirs"][d]["halves"][h]
                c_lo = col0[d]
                acc = accp.tile([128, NCH, 64], F32, tag=f"acc_{d}_{h}",
                                name=f"acc{li}_{d}_{h}")

                bidx = idx_t[li, d, h]

                def group_step(goff, frags):
                    rows = sum(f[2] for f in frags)
                    stg = stgp.tile([128, SMAX // 128, 128], BF16,
                                    tag="stg")
                    nc.gpsimd.dma_gather(
                        stg[:, 0:rows // 128, :], tab_ap,
                        bidx[:, goff // 16:(goff + rows) // 16],
                        num_idxs=rows, num_idxs_reg=rows,
                        elem_size=128, elem_step=128, single_packet=False,
                        queue_num=next_queue())
                    for stg_off, slot_off, nrows, r in frags:
                        cr = nrows // 128
                        c0 = slot_off // 128
                        s_ap = stg[:, stg_off // 128:stg_off // 128 + cr,
                                   c_lo:c_lo + 64]
                        a_ap = acc[:, c0:c0 + cr, :]
                        if r == 0:
                            nc.vector.tensor_copy(a_ap, s_ap)
                        else:
                            nc.vector.tensor_add(a_ap, a_ap, s_ap)

                def acc_step():
                    nc.sync.dma_start(
                        accd[d][hidx[h], :, :]
                        .rearrange("(c p) f -> p c f", p=128),
                        acc[:])

                steps = [(lambda goff=goff, frags=frags:
                          group_step(goff, frags))
                         for goff, frags in hinfo["groups"]]
                return steps, acc_step

            def run_interleaved(stream_steps):
                live = [list(st) for st in stream_steps]
                while live:
                    nxt = []
                    for st in live:
                        st[0]()
                        if len(st) > 1:
                            nxt.append(st[1:])
                    live = nxt

            def pair_steps(steps):
                out = []
                for i in range(0, len(steps), 2):
                    chunk = steps[i:i + 2]
                    out.append(lambda chunk=chunk: [f() for f in chunk])
                return out

            def unperm_steps(li, d, h, nat):
                """Two queue-split gathers unpermuting accd[d][h] into nat."""
                up0 = plan["dirs"][d]["halves"][h]["L"] // 16

                def mk(c0, cn):
                    def f():
                        nc.gpsimd.dma_gather(
                            nat[:, c0:c0 + cn, :], accd[d][hidx[h], :, :],
                            idx_t[li, d, h][:, up0 + c0 * 8:up0 + (c0 + cn) * 8],
                            num_idxs=cn * 128, num_idxs_reg=cn * 128,
                            elem_size=64, elem_step=64, single_packet=False,
                            queue_num=next_queue())
                    return f
                return [mk(0, 24), mk(24, NCH - 24)]

            def mean_init(d, nat, mean_tiles):
                """mean_d := nat * recip  (first half's contribution)."""
                mean = mean_tiles[d]
                rb = recip_t[d][:].unsqueeze(2).broadcast_to((128, NCH, 64))
                nc.vector.tensor_mul(mean, nat[:], rb)

            def mean_accum(d, nat, mean_tiles):
                """mean_d += nat * recip  (second half's contribution)."""
                mean = mean_tiles[d]
                rb = recip_t[d][:].unsqueeze(2).broadcast_to((128, NCH, 64))
                nc.vector.tensor_mul(nat[:], nat[:], rb)
                nc.vector.tensor_add(mean, mean, nat[:])

            def seg_widths():
                segs = []
                off = 0
                while off < NLOCP:
                    w = min(512, NLOCP - off)
                    segs.append((off, w))
                    off += w
                return segs

            def h_pass(li, get_feat, mean_pk, h_t):
                """h_packed = relu(meanT + Wr.T @ featT + bias)."""
                for off, w in seg_widths():
                    feat_ap = get_feat(off, w)
                    ps = psp.tile([128, 512], F32, tag="ps")
                    nch = w // 128
                    nc.tensor.matmul(ps[0:64, 0:w], W[f"wr_in{li}"][:],
                                     feat_ap, start=True, stop=False)
                    nc.tensor.matmul(ps[64:128, 0:w], W[f"wr_out{li}"][:],
                                     feat_ap, start=True, stop=False,
                                     tile_position=(0, 64))
                    for k in range(nch):
                        c = (off + 128 * k) // 128
                        nc.tensor.matmul(ps[:, 128 * k:128 * (k + 1)],
                                         mean_pk[:, c, :], ident_b[:],
                                         start=False, stop=k == nch - 1)
                    nc.scalar.activation(h_t[:, off:off + w], ps[:, 0:w], RELU,
                                         bias=W[f"bias_pk{li}"][:])

            def comb_pass(li, get_feat, h_t, out_cb):
                for off, w in seg_widths():
                    ps = psp.tile([128, 512], F32, tag="ps")
                    nc.tensor.matmul(ps[:, 0:w], W[f"wcx{li}"][:],
                                     get_feat(off, w), start=True, stop=False)
                    nc.tensor.matmul(ps[:, 0:w], wch_b[li][:],
                                     h_t[:, off:off + w], start=False, stop=True)
                    out_cb(off, w, ps)

            def xt_seg(off, w):
                xs = ybldp.tile([128, 512], F32, tag="xseg")
                nc.sync.dma_start(xs[:, 0:w], xt_loc[:, off:off + w])
                return xs[:, 0:w]

            # ---------------- layer 1
            mean_pk1 = meanp.tile([128, NCH, 128], BF16, tag="mean",
                                  name="mean_pk1")
            mean_tiles = {"in": mean_pk1[:, :, 0:64],
                          "out": mean_pk1[:, :, 64:128]}

            y1v = {h: y1t[h].rearrange("p k c -> (p k) c") for h in ("A", "B")}

            for d in dims:
                for h in ("A", "B"):
                    load_idx(1, d, h)
            run_interleaved([
                build_half_steps("A", 0, nc.scalar, nc.scalar, psyp, "psy",
                                 "ys"),
                build_half_steps("B", HALF, nc.sync, nc.gpsimd, psp, "ps",
                                 "ysb")])

            def layer_gathers(li, tabs, mean_tl):
                stepsA = [gather_half_steps(li, d, "A", tabs["A"])
                          for d in dims]
                stepsB = [gather_half_steps(li, d, "B", tabs["B"])
                          for d in dims]
                nb = len(stepsB[0][0])
                cut = (nb + 1) // 2
                cut2 = min(cut + 2, nb)
                # A streams at 2x rate so they finish ~when B is half done
                run_interleaved([pair_steps(st) for st, _acc in stepsA] +
                                [st[:cut] for st, _acc in stepsB])
                for _st, acc_step in stepsA:
                    acc_step()
                natA = {d: natp.tile([128, NCH, 64], F32, tag="nat",
                                     name=f"natA{li}_{d}") for d in dims}
                # 2 B-only rotations hide the accd writes the unpermutes need
                run_interleaved([st[cut:cut2] for st, _acc in stepsB])
                run_interleaved([st[cut2:] for st, _acc in stepsB] +
                                [unperm_steps(li, "in", "A", natA["in"]),
                                 unperm_steps(li, "out", "A", natA["out"])])
                for d in dims:
                    mean_init(d, natA[d], mean_tl)
                for _st, acc_step in stepsB:
                    acc_step()
                natB = {d: natp.tile([128, NCH, 64], F32, tag="nat",
                                     name=f"natB{li}_{d}") for d in dims}
                run_interleaved([unperm_steps(li, "in", "B", natB["in"]),
                                 unperm_steps(li, "out", "B", natB["out"])])
                for d in dims:
                    mean_accum(d, natB[d], mean_tl)

            layer_gathers(1, {"A": y1v["A"][:, :], "B": y1v["B"][:, :]},
                          mean_tiles)

            for d in dims:
                for h in ("A", "B"):
                    load_idx(2, d, h)
            h1_t = featp.tile([128, NLOCP], BF16, tag="hfeat")
            h_pass(1, xt_seg, mean_pk1, h1_t)
            x2_t = featp.tile([128, NLOCP], F32, tag="bigfeat")

            def l1_out(off, w, ps):
                nc.scalar.activation(x2_t[:, off:off + w], ps[:, 0:w], RELU,
                                     bias=W["cb1"][:])
            comb_pass(1, xt_seg, h1_t, l1_out)

            # y2 table slice + ONE AllGather (both dirs packed, bf16)
            for g in range((NCH + 3) // 4):
                c0 = 4 * g
                ncc = min(4, NCH - c0)
                ps = psyp.tile([128, 512], F32, tag="psy")
                for k in range(ncc):
                    nc.tensor.matmul(ps[:, 128 * k:128 * (k + 1)],
                                     x2_t[:, 128 * (c0 + k):128 * (c0 + k + 1)],
                                     W["wl_comb2"][:], start=True, stop=True)
                ys = ystgp.tile([128, 512], BF16, tag="ysb")
                nc.scalar.activation(ys[:, 0:128 * ncc], ps[:, 0:128 * ncc], COPY)
                nc.sync.dma_start(
                    y2sl[128 * c0:128 * (c0 + ncc), :]
                    .rearrange("(k p) c -> p k c", p=128),
                    ys[:, 0:128 * ncc].rearrange("p (k c) -> p k c", k=ncc))
            nc.gpsimd.collective_compute(
                "AllGather", mybir.AluOpType.bypass,
                replica_groups=[list(range(NC))],
                ins=[y2sl[0:NLOC, :]],
                outs=[y2t[ZHEAD:ZHEAD + N, :]],
            )

            # ---------------- layer 2
            mean_pk2 = meanp.tile([128, NCH, 128], BF16, tag="mean",
                                  name="mean_pk2")
            mean_tiles2 = {"in": mean_pk2[:, :, 0:64],
                           "out": mean_pk2[:, :, 64:128]}

            def l2_tab(h):
                if h == "A":
                    return y2t[0:HROWSP, :]
                return y2t[HALF:TROWS, :]
            # A-streams get 2 slots per rotation so they finish early and
            # their unpermutes overlap the B-stream tail.
            layer_gathers(2, {"A": l2_tab("A"), "B": l2_tab("B")},
                          mean_tiles2)
            h2_t = featp.tile([128, NLOCP], BF16, tag="hfeat")

            def x2_seg(off, w):
                return x2_t[:, off:off + w]
            h_pass(2, x2_seg, mean_pk2, h2_t)

            def l2_out(off, w, ps):
                x3 = ystgp.tile([128, 512], BF16, tag="x3")
                nc.scalar.activation(x3[:, 0:w], ps[:, 0:w], RELU,
                                     bias=W["cb2"][:])
                psf = psfp.tile([1, 512], F32, tag="psf")
                nc.tensor.matmul(psf[0:1, 0:w], fw_b[:], x3[:, 0:w],
                                 start=True, stop=True)
                osb = ystgp.tile([1, 512], F32, tag="osb")
                nc.vector.tensor_scalar_add(osb[0:1, 0:w],
                                            psf[0:1, 0:w], fb_t[0:1, 0:1])
                wv = min(w, NLOC - off)
                if wv > 0:
                    nc.sync.dma_start(out_t[0:1, off:off + wv], osb[0:1, 0:wv])
            comb_pass(2, x2_seg, h2_t, l2_out)

    nc.compile()
    return nc


# ------------------------------------------------------------------ interface

def _make_in_maps(plan, inputs):
    x = np.asarray(inputs["x"], dtype=np.float32)
    xt = np.zeros((128, XCOLS), dtype=np.float32)
    xt[:, :N] = np.ascontiguousarray(x.T)
    ident = np.eye(128, dtype=np.float32)

    def cat(a, b):
        return np.ascontiguousarray(
            np.concatenate([np.asarray(a, np.float32), np.asarray(b, np.float32)],
                           axis=1))

    common = {
        "xt_full": xt,
        "ident": ident,
        "wl_comb1": cat(inputs["in_Wl0"], inputs["out_Wl0"]),
        "wr_in1": np.asarray(inputs["in_Wr0"], np.float32),
        "wr_out1": np.asarray(inputs["out_Wr0"], np.float32),
        "bias_pk1": np.concatenate(
            [np.asarray(inputs["in_bl0"], np.float32),
             np.asarray(inputs["out_bl0"], np.float32)])[:, None].copy(),
        "wcx1": np.ascontiguousarray(np.asarray(inputs["comb_W0"], np.float32)[0:128]),
        "wch1": np.ascontiguousarray(np.asarray(inputs["comb_W0"], np.float32)[128:256]),
        "cb1": np.asarray(inputs["comb_b0"], np.float32)[:, None].copy(),
        "wl_comb2": cat(inputs["in_Wl1"], inputs["out_Wl1"]),
        "wr_in2": np.asarray(inputs["in_Wr1"], np.float32),
        "wr_out2": np.asarray(inputs["out_Wr1"], np.float32),
        "bias_pk2": np.concatenate(
            [np.asarray(inputs["in_bl1"], np.float32),
             np.asarray(inputs["out_bl1"], np.float32)])[:, None].copy(),
        "wcx2": np.ascontiguousarray(np.asarray(inputs["comb_W1"], np.float32)[0:128]),
        "wch2": np.ascontiguousarray(np.asarray(inputs["comb_W1"], np.float32)[128:256]),
        "cb2": np.asarray(inputs["comb_b1"], np.float32)[:, None].copy(),
        "fw": np.asarray(inputs["final_W"], np.float32).reshape(128, 1).copy(),
        "fb": np.asarray(inputs["final_b"], np.float32).reshape(1, 1).copy(),
    }
    in_maps = []
    for c in range(NC):
        m = dict(common)
        xl = np.zeros((128, NLOCP), dtype=np.float32)
        xl[:, :NLOC] = x.T[:, c * NLOC:(c + 1) * NLOC]
        m["xt_loc"] = xl
        for d in ("in", "out"):
            dinfo = plan["dirs"][d]
            rc = np.zeros((128, NCH), dtype=np.float32)
            r = dinfo["recip"][c]  # [NLOCP]
            rc[:, :] = r.reshape(NCH, 128).T
            m[f"recip_{d}"] = rc.copy()
            for h in ("A", "B"):
                hinfo = dinfo["halves"][h]
                up = _wrap_idx(hinfo["unperm"][c])
                m[f"idx1_{d}_{h}"] = np.ascontiguousarray(np.concatenate(
                    [_wrap_idx(hinfo["streams1"][c]), up], axis=1))
                m[f"idx2_{d}_{h}"] = np.ascontiguousarray(np.concatenate(
                    [_wrap_idx(hinfo["streams2"][c]), up], axis=1))
        in_maps.append(m)
    return in_maps


def kernel(**inputs):
    plan = _preprocess(np.asarray(inputs["edge_index_in"]),
                       np.asarray(inputs["edge_index_out"]))
    key = tuple(
        (d, h, tuple(plan["dirs"][d]["halves"][h]["NR"]))
        for d in ("in", "out") for h in ("A", "B"))
    if key not in _CACHE:
        _CACHE[key] = _build_program(plan)
    nc = _CACHE[key]
    in_maps = _make_in_maps(plan, inputs)
    res = bass_utils.run_bass_kernel_spmd(nc, in_maps, core_ids=list(range(NC)))
    out = np.concatenate([r["out"][0] for r in res.results])
    return out.astype(np.float32)


# revision 20
# speedup vs baseline: 1.0662x; 1.0662x over previous
"""DirectedDualSAGE (2-layer dual-direction GraphSAGE + MLP head) on 8 trn2
NeuronCores via Bass/Tile.

Sharding: nodes (dsts) block-partitioned 6250/core; each core owns all edges
whose dst lies in its shard, for both edge directions.

Per layer, lin_l(mean_j x_j) for BOTH directions is computed as
diag(1/cnt) * A * (x @ [Wl_in|Wl_out]): transform first, bf16 tables with
both directions packed per row (256B rows = the dma_gather element floor).
The segment-mean is a per-edge row gather (dma_gather, one dedicated SWDGE
queue per (direction, src-half) stream) from a DRAM table + prefix-structured
accumulation on the vector engine: dsts are sorted by descending degree so
"round r" (the r-th edge of every dst) occupies slot prefix [0, n_r) and
accumulates with one contiguous tensor_tensor add per gather fragment
(in-dir adds read gathered cols 0:64, out-dir cols 64:128). Gather indices
are int16, so edges are split by src half (< / >= 25000).

Layer 1's tables are built replicated (x is an input -> no communication) in
a PARTITION-MAJOR physical layout (logical row i at physical row
(i%128)*K + i//128, host remaps indices) so the table writes are contiguous
multi-KB runs per partition instead of 256B scatter. Layer 2's table is
built from the local x2 shard and AllGather'ed in ONE collective (both
directions packed), row-major natural layout.

Dense math runs feature-major on the tensor engine; aggregated means
(node-major) are transposed back via PE identity-matmuls accumulating into
the same PSUM as the x @ Wr term, then bias+relu on the scalar engine.

kernel(**inputs) takes full unsharded inputs, returns the full [N] output.
"""
import numpy as np

import concourse.bacc as bacc
import concourse.tile as tile
import concourse.mybir as mybir
from concourse import bass_utils

F32 = mybir.dt.float32
BF16 = mybir.dt.bfloat16
I16 = mybir.dt.int16

N = 50000
NC = 8
NLOC = N // NC            # 6250
NLOCP = 6272              # 49*128
NCH = NLOCP // 128        # 49 chunks
XCOLS = 50088             # 25000 + 49*512 (xt_full padded cols)
HALF = 25000              # src half split
ZHEAD = 128               # zero rows at table head
HROWSP = 25344            # 198*128: L1 half-table rows (partition-major)
KH = HROWSP // 128        # 198
TROWS = ZHEAD + N + 176   # 50304 rows for the AllGather'ed Y2 table
BZERO = ZHEAD + HALF      # 25128: L2 B-half zero idx (y2 row 50128 abs)
SMAX = 2560               # max rows per dma_gather call
QMAP = {("in", "A"): 0, ("in", "B"): 1, ("out", "A"): 2, ("out", "B"): 3}

_CACHE = {}


# ----------------------------------------------------------------- host prep

def _round_up(v, m):
    return (v + m - 1) // m * m


def _l1_remap(logical):
    """Partition-major physical row for L1 half tables."""
    return (logical % 128) * KH + logical // 128


def _per_core_half(src, dst, half_mask):
    out = []
    for c in range(NC):
        m = (dst // NLOC == c) & half_mask
        s = src[m]
        dloc = (dst[m] - c * NLOC).astype(np.int64)
        deg = np.bincount(dloc, minlength=NLOCP).astype(np.int64)
        perm = np.argsort(-deg, kind="stable").astype(np.int64)
        pos = np.empty(NLOCP, dtype=np.int64)
        pos[perm] = np.arange(NLOCP)
        order = np.argsort(dloc, kind="stable")
        sd = dloc[order]
        ss = s[order]
        if len(sd):
            starts = np.r_[0, 1 + np.flatnonzero(np.diff(sd))]
            group_id = np.zeros(len(sd), dtype=np.int64)
            group_id[starts[1:]] = 1
            group_id = np.cumsum(group_id)
            rank = np.arange(len(sd)) - starts[group_id]
        else:
            rank = sd
        slot = pos[sd]
        maxdeg = int(deg.max()) if len(sd) else 0
        rounds = []
        for r in range(maxdeg):
            mr = rank == r
            rounds.append((int(np.count_nonzero(mr)), slot[mr], ss[mr]))
        out.append(dict(deg=deg, pos=pos, rounds=rounds))
    return out


def _cut_groups(NR):
    L = int(sum(NR))
    groups = []  # (stream_off, [(stg_off, acc_slot_off, nrows, r)])
    r, r_off = 0, 0
    off = 0
    while off < L:
        rows = min(SMAX, L - off)
        frags = []
        done = 0
        while done < rows:
            take = min(NR[r] - r_off, rows - done)
            frags.append((done, r_off, take, r))
            done += take
            r_off += take
            if r_off == NR[r]:
                r += 1
                r_off = 0
        groups.append((off, frags))
        off += rows
    return groups


def _preprocess(edge_index_in, edge_index_out):
    plan = {"dirs": {}}
    for dname, ei in (("in", edge_index_in), ("out", edge_index_out)):
        src = ei[0].astype(np.int64)
        dst = ei[1].astype(np.int64)
        dinfo = {"halves": {}, "recip": []}
        for c in range(NC):
            m = dst // NLOC == c
            dloc = dst[m] - c * NLOC
            cnt = np.bincount(dloc, minlength=NLOCP).astype(np.float32)
            dinfo["recip"].append((1.0 / np.maximum(cnt, 1.0)).astype(np.float32))
        for hname, is_a in (("A", True), ("B", False)):
            half_mask = (src < HALF) if is_a else (src >= HALF)
            cores = _per_core_half(src, dst, half_mask)
            nrounds = max(len(ci["rounds"]) for ci in cores)
            NR = []
            for r in range(nrounds):
                mx = max((ci["rounds"][r][0] if r < len(ci["rounds"]) else 0)
                         for ci in cores)
                NR.append(_round_up(max(mx, 1), 128))
            NR[0] = NLOCP  # full first round: copy-initializes the accumulator
            streams1 = []   # layer-1 (partition-major remapped)
            streams2 = []   # layer-2 (natural + ZHEAD, shared-table offsets)
            zi2 = 0 if is_a else BZERO
            for ci in cores:
                parts1, parts2 = [], []
                for r in range(nrounds):
                    v1 = np.zeros(NR[r], dtype=np.int64)  # L1 pad -> phys 0
                    v2 = np.full(NR[r], zi2, dtype=np.int64)
                    if r < len(ci["rounds"]):
                        _, slots, ss = ci["rounds"][r]
                        rel = (ss if is_a else ss - HALF) + ZHEAD
                        v1[slots] = _l1_remap(rel)
                        v2[slots] = rel
                    parts1.append(v1)
                    parts2.append(v2)
                s1 = np.concatenate(parts1)
                s2 = np.concatenate(parts2)
                assert s1.max(initial=0) < 32768 and s2.max(initial=0) < 32768
                streams1.append(s1.astype(np.int16))
                streams2.append(s2.astype(np.int16))
            dinfo["halves"][hname] = dict(
                NR=NR, L=int(sum(NR)), streams1=streams1, streams2=streams2,
                groups=_cut_groups(NR),
                unperm=[ci["pos"].astype(np.int16) for ci in cores], is_a=is_a,
            )
        plan["dirs"][dname] = dinfo
    return plan


def _wrap_idx(idx):
    L = idx.shape[0]
    assert L % 16 == 0
    w = idx.reshape(L // 16, 16).T.astype(np.int16)
    return np.ascontiguousarray(np.tile(w, (8, 1)))


# ------------------------------------------------------------- device program

def _build_program(plan):
    nc = bacc.Bacc("TRN2", target_bir_lowering=False, debug=False,
                   num_devices=NC, num_swdge_queues=4)
    dims = ("in", "out")
    inp = {}

    def dram_in(name, shape, dt=F32):
        inp[name] = nc.dram_tensor(name, list(shape), dt, kind="ExternalInput")
        return inp[name]

    xt_full = dram_in("xt_full", [128, XCOLS], BF16)
    xt_loc = dram_in("xt_loc", [128, NLOCP], BF16)
    ident = dram_in("ident", [128, 128], BF16)
    for li in (1, 2):
        dram_in(f"wl_comb{li}", [128, 128], BF16)
        dram_in(f"wr_in{li}", [128, 64], BF16)
        dram_in(f"wr_out{li}", [128, 64], BF16)
        dram_in(f"bias_pk{li}", [128, 1])
        dram_in(f"wcx{li}", [128, 128], BF16)
        dram_in(f"wch{li}", [128, 128], BF16)
        dram_in(f"cb{li}", [128, 1])
    dram_in("fw", [128, 1], BF16)
    dram_in("fb", [1, 1])
    UPC = NLOCP // 16  # unperm idx cols
    for d in dims:
        dram_in(f"recip_{d}", [128, NCH])
        for h in ("A", "B"):
            L = plan["dirs"][d]["halves"][h]["L"]
            for li in (1, 2):
                dram_in(f"idx{li}_{d}_{h}", [128, L // 16 + UPC], I16)

    # L1 half tables: partition-major [128, KH, 128] bf16 (both dirs packed)
    y1t = {h: nc.dram_tensor(f"y1{h}", [128, KH, 128], BF16, kind="Internal")
           for h in ("A", "B")}
    y2sl = nc.dram_tensor("y2sl", [NLOCP, 128], BF16, kind="Internal")
    y2t = nc.dram_tensor("y2", [TROWS, 128], BF16, kind="Internal",
                         addr_space="Shared")
    accd = {d: nc.dram_tensor(f"accd_{d}", [2, NLOCP, 64], F32, kind="Internal")
            for d in dims}
    out_t = nc.dram_tensor("out", [1, NLOC], F32, kind="ExternalOutput")

    RELU = mybir.ActivationFunctionType.Relu
    COPY = mybir.ActivationFunctionType.Copy

    with tile.TileContext(nc) as tc:
        with tc.tile_pool(name="const", bufs=1) as cpool, \
             tc.tile_pool(name="bigidx", bufs=4) as bigidxp, \
             tc.tile_pool(name="feat", bufs=1) as featp, \
             tc.tile_pool(name="accp", bufs=1) as accp, \
             tc.tile_pool(name="meanp", bufs=1) as meanp, \
             tc.tile_pool(name="stg", bufs=6) as stgp, \
             tc.tile_pool(name="natp", bufs=2) as natp, \
             tc.tile_pool(name="ybld", bufs=2) as ybldp, \
             tc.tile_pool(name="ystg", bufs=2) as ystgp, \
             tc.tile_pool(name="small", bufs=1) as smallp, \
             tc.tile_pool(name="ps", bufs=3, space="PSUM") as psp, \
             tc.tile_pool(name="psf", bufs=1, space="PSUM") as psfp, \
             tc.tile_pool(name="psy", bufs=2, space="PSUM") as psyp:

            def load_const(name, shape, dt=F32):
                t = cpool.tile(list(shape), dt, tag=name, name=f"c_{name}")
                nc.sync.dma_start(t[:], inp[name][tuple(slice(None) for _ in shape)])
                return t

            ident_b = load_const("ident", [128, 128], BF16)
            W = {}
            for li in (1, 2):
                for nm, shp, dt in (
                        ("wl_comb", [128, 128], BF16),
                        ("wr_in", [128, 64], BF16),
                        ("wr_out", [128, 64], BF16),
                        ("bias_pk", [128, 1], F32),
                        ("wcx", [128, 128], BF16),
                        ("wch", [128, 128], BF16),
                        ("cb", [128, 1], F32)):
                    W[f"{nm}{li}"] = load_const(f"{nm}{li}", shp, dt)
            fw_t = load_const("fw", [128, 1], BF16)
            fb_t = load_const("fb", [1, 1])
            recip_t = {d: load_const(f"recip_{d}", [128, NCH]) for d in dims}
            IDXC = max(plan["dirs"][d]["halves"][h]["L"] // 16
                       for d in dims for h in ("A", "B")) + NLOCP // 16
            idx_t = {}

            def load_idx(li, d, h):
                L = plan["dirs"][d]["halves"][h]["L"]
                t = bigidxp.tile([128, IDXC], I16, tag="bigidx",
                                 name=f"bidx{li}_{d}_{h}")
                nc.scalar.dma_start(t[:, 0:L // 16 + NLOCP // 16],
                                    inp[f"idx{li}_{d}_{h}"][:, :])
                idx_t[li, d, h] = t

            zero_b = smallp.tile([128, 128], BF16, tag="zerob")
            nc.vector.memset(zero_b[:], 0.0)

            # L1 tables: zero the pad target (logical row 0 = phys row 0) and
            # the tail region beyond written data (phys rows for logical
            # >= ZHEAD+HALF+88 are never indexed; only row 0 is the pad).
            for h in ("A", "B"):
                nc.sync.dma_start(
                    y1t[h][0:1, 0:1, :].rearrange("p k c -> p (k c)"),
                    zero_b[0:1, 0:128])
            # y2 zero head+tail rows (row-major)
            def zero_rows_y2(start, nrow):
                nc.sync.dma_start(
                    y2t[start:start + nrow, :]
                    .rearrange("(k p) c -> p k c", p=nrow),
                    zero_b[0:nrow, :].rearrange("p (k c) -> p k c", k=1))
            zero_rows_y2(0, 128)
            zero_rows_y2(ZHEAD + N, 128)
            zero_rows_y2(TROWS - 48, 48)

            # ---------------- y1 table build (replicated, partition-major).
            # Loads on scalar (A) / sync (B) HWDGE, PSUM->bf16 copies on the
            # vector engine, table writes on gpsimd SWDGE: no engine carries
            # two chained stages, so the block pipeline flows at DMA rate.
            def build_half_steps(h, col0, ld_eng, w_eng, pspool, pstag,
                                 ystag):
                steps = []

                def mk(t_off):
                    def f():
                        tw = min(512, 25088 - t_off)
                        xs = ybldp.tile([128, 512], BF16, tag=f"xs{h}")
                        ld_eng.dma_start(
                            xs[:, 0:tw],
                            xt_full[:, col0 + t_off:col0 + t_off + tw])
                        ps = pspool.tile([128, 512], F32, tag=pstag)
                        for k in range(tw // 128):
                            nc.tensor.matmul(ps[:, 128 * k:128 * (k + 1)],
                                             xs[:, 128 * k:128 * (k + 1)],
                                             W["wl_comb1"][:],
                                             start=True, stop=True)
                        ys = ystgp.tile([128, 512], BF16, tag=ystag)
                        nc.vector.tensor_copy(ys[:, 0:tw], ps[:, 0:tw])
                        kk = tw // 128
                        k0 = t_off // 128 + 1  # +1: ZHEAD shifts logical rows
                        w_eng.dma_start(
                            y1t[h][:, k0:k0 + kk, :],
                            ys[:, 0:tw].rearrange("p (k c) -> p k c", k=kk))
                    return f
                for t_off in range(0, 25088, 512):
                    steps.append(mk(t_off))
                return steps

            # ---------------- shared helpers
            hidx = {"A": 0, "B": 1}
            _qctr = [0]

            def next_queue():
                q = _qctr[0] % 4
                _qctr[0] += 1
                return q
            col0 = {"in": 0, "out": 64}

            def gather_half_steps(li, d, h, tab_ap):
                """Per-group closures for one (direction, half) stream; the
                final closure writes accd. Interleave across streams to avoid
                sequencer head-of-line blocking on one queue's ring."""
                hinfo = plan["dirs"][d]["halves"][h]
                c_lo = col0[d]
                acc = accp.tile([128, NCH, 64], F32, tag=f"acc_{d}_{h}",
                                name=f"acc{li}_{d}_{h}")

                bidx = idx_t[li, d, h]

                def group_step(goff, frags):
                    rows = sum(f[2] for f in frags)
                    stg = stgp.tile([128, SMAX // 128, 128], BF16,
                                    tag="stg")
                    nc.gpsimd.dma_gather(
                        stg[:, 0:rows // 128, :], tab_ap,
                        bidx[:, goff // 16:(goff + rows) // 16],
                        num_idxs=rows, num_idxs_reg=rows,
                        elem_size=128, elem_step=128, single_packet=False,
                        queue_num=next_queue())
                    for stg_off, slot_off, nrows, r in frags:
                        cr = nrows // 128
                        c0 = slot_off // 128
                        s_ap = stg[:, stg_off // 128:stg_off // 128 + cr,
                                   c_lo:c_lo + 64]
                        a_ap = acc[:, c0:c0 + cr, :]
                        if r == 0:
                            nc.vector.tensor_copy(a_ap, s_ap)
                        else:
                            nc.vector.tensor_add(a_ap, a_ap, s_ap)

                def acc_step():
                    nc.sync.dma_start(
                        accd[d][hidx[h], :, :]
                        .rearrange("(c p) f -> p c f", p=128),
                        acc[:])

                steps = [(lambda goff=goff, frags=frags:
                          group_step(goff, frags))
                         for goff, frags in hinfo["groups"]]
                return steps, acc_step

            def run_interleaved(stream_steps):
                live = [list(st) for st in stream_steps]
                while live:
                    nxt = []
                    for st in live:
                        st[0]()
                        if len(st) > 1:
                            nxt.append(st[1:])
                    live = nxt

            def pair_steps(steps):
                out = []
                for i in range(0, len(steps), 2):
                    chunk = steps[i:i + 2]
                    out.append(lambda chunk=chunk: [f() for f in chunk])
                return out

            def unperm_steps(li, d, h, nat):
                """Two queue-split gathers unpermuting accd[d][h] into nat."""
                up0 = plan["dirs"][d]["halves"][h]["L"] // 16

                def mk(c0, cn):
                    def f():
                        nc.gpsimd.dma_gather(
                            nat[:, c0:c0 + cn, :], accd[d][hidx[h], :, :],
                            idx_t[li, d, h][:, up0 + c0 * 8:up0 + (c0 + cn) * 8],
                            num_idxs=cn * 128, num_idxs_reg=cn * 128,
                            elem_size=64, elem_step=64, single_packet=False,
                            queue_num=next_queue())
                    return f
                return [mk(0, 24), mk(24, NCH - 24)]

            def mean_init(d, nat, mean_tiles):
                """mean_d := nat * recip  (first half's contribution)."""
                mean = mean_tiles[d]
                rb = recip_t[d][:].unsqueeze(2).broadcast_to((128, NCH, 64))
                nc.vector.tensor_mul(mean, nat[:], rb)

            def mean_accum(d, nat, mean_tiles):
                """mean_d += nat * recip  (second half's contribution)."""
                mean = mean_tiles[d]
                rb = recip_t[d][:].unsqueeze(2).broadcast_to((128, NCH, 64))
                nc.vector.tensor_mul(nat[:], nat[:], rb)
                nc.vector.tensor_add(mean, mean, nat[:])

            def seg_widths():
                segs = []
                off = 0
                while off < NLOCP:
                    w = min(512, NLOCP - off)
                    segs.append((off, w))
                    off += w
                return segs

            def h_pass(li, get_feat, mean_pk, h_t):
                """h_packed = relu(meanT + Wr.T @ featT + bias)."""
                for off, w in seg_widths():
                    feat_ap = get_feat(off, w)
                    ps = psp.tile([128, 512], F32, tag="ps")
                    nch = w // 128
                    nc.tensor.matmul(ps[0:64, 0:w], W[f"wr_in{li}"][:],
                                     feat_ap, start=True, stop=False)
                    nc.tensor.matmul(ps[64:128, 0:w], W[f"wr_out{li}"][:],
                                     feat_ap, start=True, stop=False,
                                     tile_position=(0, 64))
                    for k in range(nch):
                        c = (off + 128 * k) // 128
                        nc.tensor.matmul(ps[:, 128 * k:128 * (k + 1)],
                                         mean_pk[:, c, :], ident_b[:],
                                         start=False, stop=k == nch - 1)
                    nc.scalar.activation(h_t[:, off:off + w], ps[:, 0:w], RELU,
                                         bias=W[f"bias_pk{li}"][:])

            def comb_pass(li, get_feat, h_t, out_cb):
                for off, w in seg_widths():
                    ps = psp.tile([128, 512], F32, tag="ps")
                    nc.tensor.matmul(ps[:, 0:w], W[f"wcx{li}"][:],
                                     get_feat(off, w), start=True, stop=False)
                    nc.tensor.matmul(ps[:, 0:w], W[f"wch{li}"][:],
                                     h_t[:, off:off + w], start=False, stop=True)
                    out_cb(off, w, ps)

            def xt_seg(off, w):
                xs = ybldp.tile([128, 512], BF16, tag="xseg")
                nc.sync.dma_start(xs[:, 0:w], xt_loc[:, off:off + w])
                return xs[:, 0:w]

            # ---------------- layer 1
            mean_pk1 = meanp.tile([128, NCH, 128], BF16, tag="mean",
                                  name="mean_pk1")
            mean_tiles = {"in": mean_pk1[:, :, 0:64],
                          "out": mean_pk1[:, :, 64:128]}

            y1v = {h: y1t[h].rearrange("p k c -> (p k) c") for h in ("A", "B")}

            for d in dims:
                for h in ("A", "B"):
                    load_idx(1, d, h)
            run_interleaved([
                build_half_steps("A", 0, nc.scalar, nc.scalar, psyp, "psy",
                                 "ys"),
                build_half_steps("B", HALF, nc.sync, nc.gpsimd, psp, "ps",
                                 "ysb")])

            def layer_gathers(li, tabs, mean_tl):
                stepsA = [gather_half_steps(li, d, "A", tabs["A"])
                          for d in dims]
                stepsB = [gather_half_steps(li, d, "B", tabs["B"])
                          for d in dims]
                nb = len(stepsB[0][0])
                cut = (nb + 1) // 2
                cut2 = min(cut + 2, nb)
                # A streams at 2x rate so they finish ~when B is half done
                run_interleaved([pair_steps(st) for st, _acc in stepsA] +
                                [st[:cut] for st, _acc in stepsB])
                for _st, acc_step in stepsA:
                    acc_step()
                natA = {d: natp.tile([128, NCH, 64], F32, tag="nat",
                                     name=f"natA{li}_{d}") for d in dims}
                # 2 B-only rotations hide the accd writes the unpermutes need
                run_interleaved([st[cut:cut2] for st, _acc in stepsB])
                run_interleaved([st[cut2:] for st, _acc in stepsB] +
                                [unperm_steps(li, "in", "A", natA["in"]),
                                 unperm_steps(li, "out", "A", natA["out"])])
                for d in dims:
                    mean_init(d, natA[d], mean_tl)
                for _st, acc_step in stepsB:
                    acc_step()
                natB = {d: natp.tile([128, NCH, 64], F32, tag="nat",
                                     name=f"natB{li}_{d}") for d in dims}
                run_interleaved([unperm_steps(li, "in", "B", natB["in"]),
                                 unperm_steps(li, "out", "B", natB["out"])])
                for d in dims:
                    mean_accum(d, natB[d], mean_tl)

            layer_gathers(1, {"A": y1v["A"][:, :], "B": y1v["B"][:, :]},
                          mean_tiles)

            for d in dims:
                for h in ("A", "B"):
                    load_idx(2, d, h)
            h1_t = featp.tile([128, NLOCP], BF16, tag="hfeat")
            h_pass(1, xt_seg, mean_pk1, h1_t)
            x2_t = featp.tile([128, NLOCP], BF16, tag="bigfeat")

            def l1_out(off, w, ps):
                nc.scalar.activation(x2_t[:, off:off + w], ps[:, 0:w], RELU,
                                     bias=W["cb1"][:])
            comb_pass(1, xt_seg, h1_t, l1_out)

            # y2 table slice + ONE AllGather (both dirs packed, bf16)
            for g in range((NCH + 3) // 4):
                c0 = 4 * g
                ncc = min(4, NCH - c0)
                ps = psyp.tile([128, 512], F32, tag="psy")
                for k in range(ncc):
                    nc.tensor.matmul(ps[:, 128 * k:128 * (k + 1)],
                                     x2_t[:, 128 * (c0 + k):128 * (c0 + k + 1)],
                                     W["wl_comb2"][:], start=True, stop=True)
                ys = ystgp.tile([128, 512], BF16, tag="ysb")
                nc.scalar.activation(ys[:, 0:128 * ncc], ps[:, 0:128 * ncc], COPY)
                nc.sync.dma_start(
                    y2sl[128 * c0:128 * (c0 + ncc), :]
                    .rearrange("(k p) c -> p k c", p=128),
                    ys[:, 0:128 * ncc].rearrange("p (k c) -> p k c", k=ncc))
            nc.gpsimd.collective_compute(
                "AllGather", mybir.AluOpType.bypass,
                replica_groups=[list(range(NC))],
                ins=[y2sl[0:NLOC, :]],
                outs=[y2t[ZHEAD:ZHEAD + N, :]],
            )

            # ---------------- layer 2
            mean_pk2 = meanp.tile([128, NCH, 128], BF16, tag="mean",
                                  name="mean_pk2")
            mean_tiles2 = {"in": mean_pk2[:, :, 0:64],
                           "out": mean_pk2[:, :, 64:128]}

            def l2_tab(h):
                if h == "A":
                    return y2t[0:HROWSP, :]
                return y2t[HALF:TROWS, :]
            # A-streams get 2 slots per rotation so they finish early and
            # their unpermutes overlap the B-stream tail.
            layer_gathers(2, {"A": l2_tab("A"), "B": l2_tab("B")},
                          mean_tiles2)
            h2_t = featp.tile([128, NLOCP], BF16, tag="hfeat")

            def x2_seg(off, w):
                return x2_t[:, off:off + w]
            h_pass(2, x2_seg, mean_pk2, h2_t)

            def l2_out(off, w, ps):
                x3 = ystgp.tile([128, 512], BF16, tag="x3")
                nc.scalar.activation(x3[:, 0:w], ps[:, 0:w], RELU,
                                     bias=W["cb2"][:])
                psf = psfp.tile([1, 512], F32, tag="psf")
                nc.tensor.matmul(psf[0:1, 0:w], fw_t[:], x3[:, 0:w],
                                 start=True, stop=True)
                osb = ystgp.tile([1, 512], F32, tag="osb")
                nc.vector.tensor_scalar_add(osb[0:1, 0:w],
                                            psf[0:1, 0:w], fb_t[0:1, 0:1])
                wv = min(w, NLOC - off)
                if wv > 0:
                    nc.sync.dma_start(out_t[0:1, off:off + wv], osb[0:1, 0:wv])
            comb_pass(2, x2_seg, h2_t, l2_out)

    nc.compile()
    return nc


# ------------------------------------------------------------------ interface

def _make_in_maps(plan, inputs):
    import ml_dtypes
    BF = ml_dtypes.bfloat16
    x = np.asarray(inputs["x"], dtype=np.float32)
    xt = np.zeros((128, XCOLS), dtype=BF)
    xt[:, :N] = np.ascontiguousarray(x.T).astype(BF)
    ident = np.eye(128, dtype=BF)

    def cat(a, b):
        return np.ascontiguousarray(
            np.concatenate([np.asarray(a, np.float32), np.asarray(b, np.float32)],
                           axis=1)).astype(BF)

    common = {
        "xt_full": xt,
        "ident": ident,
        "wl_comb1": cat(inputs["in_Wl0"], inputs["out_Wl0"]),
        "wr_in1": np.asarray(inputs["in_Wr0"], np.float32).astype(BF),
        "wr_out1": np.asarray(inputs["out_Wr0"], np.float32).astype(BF),
        "bias_pk1": np.concatenate(
            [np.asarray(inputs["in_bl0"], np.float32),
             np.asarray(inputs["out_bl0"], np.float32)])[:, None].copy(),
        "wcx1": np.ascontiguousarray(
            np.asarray(inputs["comb_W0"], np.float32)[0:128]).astype(BF),
        "wch1": np.ascontiguousarray(
            np.asarray(inputs["comb_W0"], np.float32)[128:256]).astype(BF),
        "cb1": np.asarray(inputs["comb_b0"], np.float32)[:, None].copy(),
        "wl_comb2": cat(inputs["in_Wl1"], inputs["out_Wl1"]),
        "wr_in2": np.asarray(inputs["in_Wr1"], np.float32).astype(BF),
        "wr_out2": np.asarray(inputs["out_Wr1"], np.float32).astype(BF),
        "bias_pk2": np.concatenate(
            [np.asarray(inputs["in_bl1"], np.float32),
             np.asarray(inputs["out_bl1"], np.float32)])[:, None].copy(),
        "wcx2": np.ascontiguousarray(
            np.asarray(inputs["comb_W1"], np.float32)[0:128]).astype(BF),
        "wch2": np.ascontiguousarray(
            np.asarray(inputs["comb_W1"], np.float32)[128:256]).astype(BF),
        "cb2": np.asarray(inputs["comb_b1"], np.float32)[:, None].copy(),
        "fw": np.asarray(inputs["final_W"], np.float32).reshape(128, 1)
              .astype(BF).copy(),
        "fb": np.asarray(inputs["final_b"], np.float32).reshape(1, 1).copy(),
    }
    in_maps = []
    for c in range(NC):
        m = dict(common)
        xl = np.zeros((128, NLOCP), dtype=BF)
        xl[:, :NLOC] = x.T[:, c * NLOC:(c + 1) * NLOC].astype(BF)
        m["xt_loc"] = xl
        for d in ("in", "out"):
            dinfo = plan["dirs"][d]
            rc = np.zeros((128, NCH), dtype=np.float32)
            r = dinfo["recip"][c]  # [NLOCP]
            rc[:, :] = r.reshape(NCH, 128).T
            m[f"recip_{d}"] = rc.copy()
            for h in ("A", "B"):
                hinfo = dinfo["halves"][h]
                up = _wrap_idx(hinfo["unperm"][c])
                m[f"idx1_{d}_{h}"] = np.ascontiguousarray(np.concatenate(
                    [_wrap_idx(hinfo["streams1"][c]), up], axis=1))
                m[f"idx2_{d}_{h}"] = np.ascontiguousarray(np.concatenate(
                    [_wrap_idx(hinfo["streams2"][c]), up], axis=1))
        in_maps.append(m)
    return in_maps


def kernel(**inputs):
    plan = _preprocess(np.asarray(inputs["edge_index_in"]),
                       np.asarray(inputs["edge_index_out"]))
    key = tuple(
        (d, h, tuple(plan["dirs"][d]["halves"][h]["NR"]))
        for d in ("in", "out") for h in ("A", "B"))
    if key not in _CACHE:
        _CACHE[key] = _build_program(plan)
    nc = _CACHE[key]
    in_maps = _make_in_maps(plan, inputs)
    res = bass_utils.run_bass_kernel_spmd(nc, in_maps, core_ids=list(range(NC)))
    out = np.concatenate([r["out"][0] for r in res.results])
    return out.astype(np.float32)
